# revision 17
# baseline (speedup 1.0000x reference)
"""Trainium2 Bass kernel for nn_MultiHeadAttentionLayer (edge-wise MHA with
global softmax over the edge dimension).

v2 strategy (8 NeuronCores, data-parallel over edges):
  - Host shards E=250000 edges into 8 shards of 31250, zero-padded to 31744
    (62 chunks x 512), pre-transposed so features land on SBUF partitions,
    cast to bf16.  The edge-attr stream is repacked 2-up: partitions 0-31 =
    even chunk's 32 features, partitions 32-63 = odd chunk's, so the two
    32-contract edge matmuls of a chunk pair run CONCURRENTLY in the PE
    array via tile_position row-packing (row groups 0 and 32).
  - A1 per chunk: Q = wq.T@xiT (PSUM), KE = wk.T@xjT (+packed we matmul).
    The K/edge bias (bk+be) is folded into the KE PSUM->SBUF copy's bias
    operand (ACT activation / DVE tensor_scalar, alternating for balance).
    P = (Q+bq)*KE via DVE stt; S = HsumRep.T@P deferred 2 chunks;
    exp(S/4) -> resident SBUF bf16 e_full + per-chunk Z partials via the
    ACT accumulator.  x_j streams into a resident SBUF buffer.
    Pad cols produce a per-head constant exp(headsum(bq*bke)/4) which the
    host folds into a [128,1] correction vector subtracted after the AR.
  - Z chain: DVE free-axis reduce of zparts, then the d_zin DMA is issued
    from the VECTOR queue (in program order right after the reduce; the
    sync queue had a ~14us descriptor backlog in v1 that delayed the
    collective trigger).  AllReduce(add) on the GPSIMD queue (collectives
    are restricted to Pool/DMA queues; it blocks its queue until done).
  - While the AR is in flight, phase A2: V = wv.T@xjT; U = (V+bv)*e_full
    in place over xj_full.  1/3 of chunks DVE stt straight from PSUM; 2/3
    ACT copy (V+bv -> bf16) + DVE all-bf16 multiply at 2x rate.
    A handful of dummy matmuls after A2 keep the PE HAM-warm through any
    residual AR stall (v1 restarted pass B cold at 1.2GHz for ~4us).
  - Post-AR: zsum load on the idle TENSOR queue, pad-vector subtract,
    reciprocal, wo2 = wo * (1/Z) per-head row scale.
  - Pass B per chunk pair: out = wo2.T@U + bo -> fp16; the two 512-wide
    halves of the PSUM->SBUF copy run on ACT and DVE in PARALLEL; the
    output DMA issue rotates across sync/gpsimd/tensor queues (v1 issued
    all stores on sync at 665ns each, near the phase's DMA floor).
  - Host gathers and transposes back to [E, 128].
"""
import os
import sys

for _p in ("/opt/trn_rl_repo", "/root/.axon_site/_ro/trn_rl_repo"):
    if os.path.isdir(_p) and _p not in sys.path:
        sys.path.append(_p)

import numpy as np
import ml_dtypes
import concourse.bacc as bacc
import concourse.tile as tile
import concourse.mybir as mybir
from concourse.bass_utils import run_bass_kernel_spmd

F32 = mybir.dt.float32
BF16 = mybir.dt.bfloat16
AF = mybir.ActivationFunctionType
ALU = mybir.AluOpType
BF = ml_dtypes.bfloat16

E_FULL = 250000
NCORES = 8
ES = E_FULL // NCORES          # 31250 edges per core
CH = 512                       # chunk size (PSUM bank width)
NCH = (ES + CH - 1) // CH      # 62 chunks
NG = NCH // 2                  # 31 chunk pairs (groups)
EP = NCH * CH                  # 31744 padded edges per core
D = 128
NH = 8
DK = 16
XW = 4096                      # DMA batch width (8 chunks)
NPAIR = NCH // 2               # 31 output pairs
NPAD = EP - ES                 # 494 pad cols per core

_CACHE = {}


def _build():
    if "nc" in _CACHE:
        return _CACHE["nc"]

    nc = bacc.Bacc(num_devices=NCORES)

    t_xiT = nc.dram_tensor("xiT", [D, EP], BF16, kind="ExternalInput")
    t_xjT = nc.dram_tensor("xjT", [D, EP], BF16, kind="ExternalInput")
    t_eaP = nc.dram_tensor("eaP", [64, EP // 2], BF16, kind="ExternalInput")
    t_pkb = nc.dram_tensor("pkb", [D, 768], BF16, kind="ExternalInput")
    t_pkf = nc.dram_tensor("pkf", [D, 8], F32, kind="ExternalInput")
    t_out = nc.dram_tensor("outT", [D, EP], mybir.dt.float16, kind="ExternalOutput")

    with tile.TileContext(nc) as tc:
        with (
            tc.tile_pool(name="per", bufs=1) as per,      # persistent
            tc.tile_pool(name="wk", bufs=2) as wk,        # streaming loads
            tc.tile_pool(name="mid", bufs=2) as mid,      # intermediates
            tc.tile_pool(name="dram", bufs=1, space="DRAM") as dram,
        ):
            s_pkb = per.tile([D, 768], BF16)
            nc.scalar.dma_start(s_pkb[:], t_pkb[:])
            s_wq = s_pkb[:, 0:128]
            s_wk = s_pkb[:, 128:256]
            s_wv = s_pkb[:, 256:384]
            s_wo = s_pkb[:, 384:512]
            s_wea = s_pkb[:, 512:640]        # we at parts 0-31 AND 32-63
            s_hrep = s_pkb[:, 640:768]       # HsumRep [f, hd]

            s_pkf = per.tile([D, 8], F32)
            nc.scalar.dma_start(s_pkf[:], t_pkf[:])
            s_bq = s_pkf[:, 0:1]
            s_bke = s_pkf[:, 1:2]
            s_bv = s_pkf[:, 2:3]
            s_bo = s_pkf[:, 3:4]
            s_padv = s_pkf[:, 4:5]           # 8*NPAD*exp(headsum(bq*bke)/4)

            xj_full = per.tile([D, EP], BF16)    # resident x_j^T (later U)
            e_full = per.tile([D, EP], BF16)     # resident exp, replicated
            zparts = per.tile([D, NCH], F32)     # per-chunk Z partials

            # ---------------- phase A1: scores ----------------
            psA_ctx = tc.tile_pool(name="psA", bufs=1, space="PSUM")
            psA = psA_ctx.__enter__()
            # Warm-up AllReduce: pays the ~11.5us CC arming cost and the
            # first rendezvous early, so the real AR triggers in ~1us.
            # It sits AFTER the framework's gpsimd const-memsets (program
            # order), so it cannot block A1's constant tiles.
            warm = per.tile([D, CH], BF16)
            nc.gpsimd.memset(warm[:], 0.0)
            d_w1 = dram.tile([8, 1], F32)
            d_w2 = dram.tile([8, 1], F32)
            nc.scalar.dma_start(d_w1[:], s_pkf[0:8, 7:8])
            nc.gpsimd.collective_compute(
                "AllReduce", ALU.add,
                replica_groups=[list(range(NCORES))],
                ins=[d_w1.opt()],
                outs=[d_w2.opt()],
            )
            # Early exp-table load: a tiny dummy activation so the ACT
            # table DMA happens during the first input DMA wait, plus PE
            # pre-warm matmuls so the HAM un-throttles before the stream.
            dume = per.tile([D, 1], BF16)
            nc.scalar.activation(dume[:], warm[:, 0:1], AF.Exp,
                                 bias=0.0, scale=0.25)
            p_warm = psA.tile([D, CH], F32, tag="pq", bufs=3, name="p_warm")
            for i in range(12):
                nc.tensor.matmul(p_warm[:], warm[:, 128 * (i % 2):128 * (i % 2) + 128],
                                 warm[:], start=True, stop=True)
            # scratch target for the gpsimd keep-awake no-ops below
            s_gnop = per.tile([1, 4], BF16)

            pchain = {}      # P tiles for the deferred S matmuls
            kecnt = [0]

            def do_s(c):
                ps8 = psA.tile([D, CH], F32, tag="ps8", bufs=2,
                               name=f"ps8_{c}")
                nc.tensor.matmul(ps8[:], s_hrep, pchain.pop(c)[:],
                                 start=True, stop=True)
                sl1 = slice(c * CH, (c + 1) * CH)
                nc.scalar.activation(e_full[:, sl1], ps8[:], AF.Exp,
                                     bias=0.0, scale=0.25,
                                     accum_out=zparts[:, c:c + 1])

            def elem(c, p_q, p_ke):
                # KE -> SBUF copy with (bk+be) bias folded in; alternate
                # ACT (4/9) / DVE (5/9) for engine balance
                s_ke = mid.tile([D, CH], BF16, tag="ke", bufs=6)
                k = kecnt[0]
                kecnt[0] += 1
                if k % 9 < 4:
                    nc.scalar.activation(s_ke[:], p_ke[:], AF.Identity,
                                         bias=s_bke, scale=1.0)
                else:
                    nc.vector.tensor_scalar(s_ke[:], p_ke[:], s_bke, None,
                                            op0=ALU.add)
                # P = (Q + bq) * KE (DVE)
                s_p = mid.tile([D, CH], BF16, tag="p", bufs=6)
                nc.vector.scalar_tensor_tensor(s_p[:], p_q[:], s_bq, s_ke[:],
                                               op0=ALU.add, op1=ALU.mult)
                pchain[c] = s_p

            for g in range(NG):
                pq = {}
                pk = {}
                for c in (2 * g, 2 * g + 1):
                    sl = slice(c * CH, (c + 1) * CH)
                    if c % (XW // CH) == 0:
                        w = min(XW, EP - c * CH)
                        s_xi = wk.tile([D, XW], BF16, tag="xi", bufs=3)
                        s_ea = wk.tile([64, XW // 2], BF16, tag="ea", bufs=3)
                        if c == 0:
                            pieces = [(0, CH), (CH, 2 * CH), (2 * CH, w)]
                        else:
                            pieces = [(0, w)]
                        for lo, hi in pieces:
                            psl = slice(c * CH + lo, c * CH + hi)
                            nc.sync.dma_start(s_xi[:, lo:hi], t_xiT[:, psl])
                            nc.sync.dma_start(
                                s_ea[:, lo // 2:hi // 2],
                                t_eaP[:, (c * CH + lo) // 2:(c * CH + hi) // 2])
                            nc.sync.dma_start(xj_full[:, psl], t_xjT[:, psl])
                    xsl = slice((c % (XW // CH)) * CH, (c % (XW // CH)) * CH + CH)

                    p_q = psA.tile([D, CH], F32, tag="pq", bufs=3)
                    nc.tensor.matmul(p_q[:], s_wq, s_xi[:, xsl], start=True, stop=True)
                    p_ke = psA.tile([D, CH], F32, tag="pke", bufs=3)
                    nc.tensor.matmul(p_ke[:], s_wk, xj_full[:, sl], start=True, stop=False)
                    pq[c] = (p_q, s_xi, xsl)
                    pk[c] = p_ke
                # packed pair of 32-contract edge matmuls: row groups 0/32
                # run concurrently in the PE array
                gcol = slice(g * CH, (g + 1) * CH)
                esl = slice((g % (XW // CH // 2)) * CH, (g % (XW // CH // 2)) * CH + CH)
                nc.tensor.matmul(pk[2 * g][:], s_wea[0:32, :], s_ea[0:32, esl],
                                 start=False, stop=True, tile_position=(0, 0))
                nc.tensor.matmul(pk[2 * g + 1][:], s_wea[32:64, :], s_ea[32:64, esl],
                                 start=False, stop=True, tile_position=(32, 0))
                for c in (2 * g, 2 * g + 1):
                    elem(c, pq[c][0], pk[c])
                # S matmul + exp deferred one group (2 chunks)
                if g >= 1:
                    do_s(2 * g - 2)
                    do_s(2 * g - 1)
                    # keep the gpsimd queue awake through A1: a sleeping
                    # queue took ~8.7us to wake for the collective trigger.
                    # Reads xj_full (paced by the input DMA batches), NOT
                    # e_full -- an e_full read serialized against the next
                    # exp's write and cost A1 ~4us.
                    nc.gpsimd.tensor_scalar(
                        s_gnop[:], xj_full[0:1, (2 * g - 2) * CH:(2 * g - 2) * CH + 4],
                        0.0, None, op0=ALU.add)
            do_s(NCH - 2)
            do_s(NCH - 1)

            # ---------------- global Z (AllReduce, hidden under A2) -------
            s_zl = per.tile([D, 1], F32)
            nc.vector.tensor_reduce(s_zl[:], zparts[:],
                                    axis=mybir.AxisListType.X, op=ALU.add)
            d_zin = dram.tile([D, 1], F32)
            d_zout = dram.tile([D, 1], F32)
            # software-DGE store on the gpsimd queue itself: the collective
            # right behind it then needs no DMA-completion semaphore (a
            # DRAM-write completion sem cost ~10us of trigger latency)
            nc.gpsimd.dma_start(d_zin[:], s_zl[:])
            nc.gpsimd.collective_compute(
                "AllReduce", ALU.add,
                replica_groups=[list(range(NCORES))],
                ins=[d_zin.opt()],
                outs=[d_zout.opt()],
            )

            psA_ctx.__exit__(None, None, None)
            ps2_ctx = tc.tile_pool(name="ps2", bufs=1, space="PSUM")
            ps2 = ps2_ctx.__enter__()

            # ---------------- phase A2: V and U (runs during the AR) ------
            # U = (V + bv) * e, in place over the consumed x_j chunk.
            # Post-A1 is ACT+DVE-work-bound, so the split minimizes
            # max(ACT, DVE) FIFO totals: stt chunks pay 691ns on DVE only,
            # copy+mult pairs pay 713 ACT + 351 DVE, and the last 21
            # chunks' multiplies go to GPSIMD (free once the AR retires).
            s_zsum = per.tile([D, 1], F32)
            s_zc = per.tile([D, 1], F32)
            s_chd = per.tile([D, 1], F32)
            s_wo2 = per.tile([D, D], BF16)
            # gpsimd FIFO: these sit right behind the collective, so the
            # Z -> wo2 chain starts the instant the AR retires (the DVE
            # reciprocal below is the one non-gpsimd link)
            nc.gpsimd.dma_start(s_zsum[:], d_zout[:])
            # pad cols contribute exp(headsum(bq*bke)/4) each: subtract
            nc.gpsimd.tensor_tensor(s_zc[:], s_zsum[:], s_padv,
                                    op=ALU.subtract)
            nc.gpsimd.tensor_scalar(s_wo2[:], s_wo, s_chd[:], None,
                                    op0=ALU.mult)
            for c in range(NCH):
                sl = slice(c * CH, (c + 1) * CH)
                p_v = ps2.tile([D, CH], F32, tag="pv", bufs=4)
                nc.tensor.matmul(p_v[:], s_wv, xj_full[:, sl], start=True, stop=True)
                if c >= NCH - 21:
                    # deeper ring: gpsimd is AR-blocked while ACT runs
                    # ahead filling these
                    v_sb = mid.tile([D, CH], BF16, tag="vsbg", bufs=12)
                    nc.scalar.activation(v_sb[:], p_v[:], AF.Identity,
                                         bias=s_bv, scale=1.0)
                    nc.gpsimd.tensor_tensor(xj_full[:, sl], e_full[:, sl],
                                            v_sb[:], op=ALU.mult)
                elif c % 3 == 0:
                    v_sb = mid.tile([D, CH], BF16, tag="vsb", bufs=6)
                    nc.scalar.activation(v_sb[:], p_v[:], AF.Identity,
                                         bias=s_bv, scale=1.0)
                    nc.vector.tensor_tensor(xj_full[:, sl], e_full[:, sl],
                                            v_sb[:], op=ALU.mult)
                else:
                    nc.vector.scalar_tensor_tensor(xj_full[:, sl], p_v[:],
                                                   s_bv, e_full[:, sl],
                                                   op0=ALU.add, op1=ALU.mult)
                if c == 34:
                    # reciprocal placed mid-FIFO: DVE reaches this slot at
                    # about the time the AR lands, so wo2 is ready before
                    # pass B's first matmul instead of after A2 drains
                    nc.vector.reciprocal(s_chd[:], s_zc[:])

            # dummy matmuls to keep the PE HAM-warm through any residual
            # AR stall (they have no deps and run right after A2's V's)
            p_w2 = ps2.tile([D, CH], F32, tag="pv", bufs=4, name="p_w2")
            for i in range(12):
                nc.tensor.matmul(p_w2[:], warm[:, 0:128], warm[:],
                                 start=True, stop=True)

            ps2_ctx.__exit__(None, None, None)
            psB_ctx = tc.tile_pool(name="psB", bufs=1, space="PSUM")
            psB = psB_ctx.__enter__()

            # ---------------- pass B: output (pair-granularity) -----------
            # per pair: 2 matmuls; the two 512-wide PSUM->SBUF copy halves
            # run on ACT and DVE in parallel; stores are 2 pairs (512KB)
            # per descriptor on the otherwise-idle sync queue
            s_o = None
            for p in range(NPAIR):
                p_o = psB.tile([D, 2 * CH], F32, tag="po", bufs=4,
                               name=f"po_{p}")
                for h in range(2):
                    hs = slice((2 * p + h) * CH, (2 * p + h + 1) * CH)
                    nc.tensor.matmul(p_o[:, h * CH:(h + 1) * CH],
                                     s_wo2[:],
                                     xj_full[:, hs], start=True, stop=True)
                if p % 2 == 0:
                    s_o = mid.tile([D, 4 * CH], mybir.dt.float16, tag="o",
                                   bufs=3)
                po2 = (p % 2) * 2 * CH
                nc.scalar.activation(s_o[:, po2:po2 + CH], p_o[:, 0:CH],
                                     AF.Identity, bias=s_bo, scale=1.0)
                nc.vector.tensor_scalar(s_o[:, po2 + CH:po2 + 2 * CH],
                                        p_o[:, CH:2 * CH],
                                        s_bo, None, op0=ALU.add)
                if p % 2 == 1 or p == NPAIR - 1:
                    t0c = (p // 2) * 4 * CH
                    w = po2 + 2 * CH
                    nc.sync.dma_start(t_out[:, t0c:t0c + w], s_o[:, 0:w])
            psB_ctx.__exit__(None, None, None)

    nc.compile()
    _CACHE["nc"] = nc
    return nc


def _pack_constants(wq, bq, wk, bk, wv, bv, we, be, wo, bo):
    HsumRep = np.zeros((D, D), np.float32)   # [f, hd] = (head(f)==head(hd))
    for f in range(D):
        h = f // DK
        HsumRep[f, h * DK:(h + 1) * DK] = 1.0
    pkb = np.zeros((D, 768), np.float32)
    pkb[:, 0:128] = wq
    pkb[:, 128:256] = wk
    pkb[:, 256:384] = wv
    pkb[:, 384:512] = wo
    pkb[0:32, 512:640] = we                  # row-group 0 copy
    pkb[32:64, 512:640] = we                 # row-group 32 copy
    pkb[:, 640:768] = HsumRep
    bke = bk + be
    pkf = np.zeros((D, 8), np.float32)
    pkf[:, 0] = bq
    pkf[:, 1] = bke
    pkf[:, 2] = bv
    pkf[:, 3] = bo
    # pad cols: Q=bq, KE=bke -> e_pad[f] = exp(headsum(bq*bke)/4) per head
    prod = bq * bke
    hsum = prod.reshape(NH, DK).sum(axis=1)
    cpad = np.exp(hsum / 4.0)
    pkf[:, 4] = NCORES * NPAD * np.repeat(cpad, DK)
    return pkb.astype(BF), pkf


def _run(inputs, trace=False):
    x_i = np.asarray(inputs["x_i"], np.float32)
    x_j = np.asarray(inputs["x_j"], np.float32)
    ea = np.asarray(inputs["edge_attr"], np.float32)
    pkb, pkf = _pack_constants(
        np.asarray(inputs["wq"], np.float32), np.asarray(inputs["bq"], np.float32),
        np.asarray(inputs["wk"], np.float32), np.asarray(inputs["bk"], np.float32),
        np.asarray(inputs["wv"], np.float32), np.asarray(inputs["bv"], np.float32),
        np.asarray(inputs["we"], np.float32), np.asarray(inputs["be"], np.float32),
        np.asarray(inputs["wo"], np.float32), np.asarray(inputs["bo"], np.float32),
    )

    in_maps = []
    for c in range(NCORES):
        sl = slice(c * ES, (c + 1) * ES)
        xiT = np.zeros((D, EP), BF)
        xiT[:, :ES] = x_i[sl].T.astype(BF)
        xjT = np.zeros((D, EP), BF)
        xjT[:, :ES] = x_j[sl].T.astype(BF)
        # 2-up repack: group g cols, parts 0-31 = chunk 2g, 32-63 = 2g+1
        eaT = np.zeros((NCH, 32, CH), BF)
        eav = np.zeros((EP, 32), np.float32)
        eav[:ES] = ea[sl]
        eaT[:] = eav.reshape(NCH, CH, 32).transpose(0, 2, 1).astype(BF)
        eaP = np.zeros((64, EP // 2), BF)
        eaP[0:32] = eaT[0::2].transpose(1, 0, 2).reshape(32, EP // 2)
        eaP[32:64] = eaT[1::2].transpose(1, 0, 2).reshape(32, EP // 2)
        in_maps.append(dict(xiT=xiT, xjT=xjT, eaP=eaP, pkb=pkb, pkf=pkf))

    nc = _build()
    res = run_bass_kernel_spmd(nc, in_maps, list(range(NCORES)), trace=trace)

    out = np.empty((E_FULL, D), np.float32)
    for c in range(NCORES):
        sl = slice(c * ES, (c + 1) * ES)
        out[sl] = res.results[c]["outT"][:, :ES].T.astype(np.float32)
    return out, res.exec_time_ns


def kernel(**inputs) -> np.ndarray:
    return _run(inputs)[0]
